# revision 1
# baseline (speedup 1.0000x reference)
"""Causal self-attention (B=4, T=2048, C=1024, H=16) on 8 TRN2 NeuronCores.

Sharding: hybrid batch x head tensor-parallel. Core c handles batch b = c//2
and heads [8*(c%2) : 8*(c%2)+8]. Each core computes QKV for its 8 heads over
its batch, full causal attention for those heads, and a *partial* c_proj
(contribution of its 8 heads to all 2048 tokens of its batch). The host
unshards by summing the two partial outputs of each batch pair (the c_proj
all-reduce of the pair, done at gather time); b_proj is added on-device by
the even core of each pair.

Device kernel layout choices (per core):
  - x is fed transposed (xt [C, T]) so QKV can be computed directly in the
    orientations attention wants: Q_T/K_T as [head-dim, token] (lhsT = w
    chunks, rhs = xt), V as [token, head-dim] (lhsT = xt chunks, rhs = w_v).
  - scores_T[k, q] = K_T_slice.T @ Q_T (contract over d=64). Softmax runs
    without max-subtraction (|score*scale| <= ~6 for this distribution, exp
    is safe in fp32); causal masking is a memset of fully-masked column
    ranges plus one triangular-mask multiply on the diagonal 128x128 block.
  - attn@V with lhsT = [V | ones] so PSUM row 64 accumulates the softmax
    denominators for free; normalization divides y_T by that row
    (reciprocal + gpsimd partition_broadcast + one DVE multiply).
  - All matmuls run as float32r (full-rate fp32 on the PE for N>=256).
"""

import numpy as np

import concourse.bass as bass
import concourse.mybir as mybir
import concourse.tile as tile
from concourse import bacc
from concourse.bass_utils import run_bass_kernel_spmd

B, T, C = 4, 2048, 1024
H = 16          # total heads
HL = 8          # heads per core
D = 64          # head dim
P = 128
W = 512         # matmul moving-dim window
NW = T // W     # 4 q windows
KB = T // P     # 16 k blocks
NCHUNK = C // P  # 8 contraction chunks over C
PAIRS = HL // 2  # 4 head-pairs (2 heads per 128-partition tile)
F32 = mybir.dt.float32
F32R = mybir.dt.float32r
EXP = mybir.ActivationFunctionType.Exp
N_CORES = 8

_CACHE = {}
LAST_RESULTS = None


def _r(ap):
    # tiles feeding matmuls are allocated as float32r already
    return ap


def build_nc():
    if "nc" in _CACHE:
        return _CACHE["nc"]
    nc = bacc.Bacc(
        "TRN2", target_bir_lowering=False, debug=False, num_devices=N_CORES
    )

    xt = nc.dram_tensor("xt", [C, T], F32R, kind="ExternalInput")
    wqk = nc.dram_tensor("wqk", [C, C], F32R, kind="ExternalInput")
    wv = nc.dram_tensor("wv", [C, HL * D], F32R, kind="ExternalInput")
    bqk = nc.dram_tensor("bqk", [P, 2 * PAIRS], F32, kind="ExternalInput")
    bv = nc.dram_tensor("bv", [P, HL * D], F32, kind="ExternalInput")
    wp = nc.dram_tensor("wp", [HL * D, C], F32R, kind="ExternalInput")
    bpr = nc.dram_tensor("bpr", [P, C], F32, kind="ExternalInput")
    trimask = nc.dram_tensor("trimask", [P, P], F32, kind="ExternalInput")
    onesd = nc.dram_tensor("onesd", [P, P], F32R, kind="ExternalInput")
    out = nc.dram_tensor("out", [T, C], F32, kind="ExternalOutput")

    with tile.TileContext(nc) as tc, nc.allow_low_precision(
        reason="float32r tiles for full-rate fp32 PE matmuls"
    ):
        with tc.tile_pool(name="consts", bufs=1) as consts:
            tri_t = consts.tile([P, P], F32)
            nc.sync.dma_start(tri_t[:], trimask[:])
            bqk_t = consts.tile([P, 2 * PAIRS], F32)
            nc.sync.dma_start(bqk_t[:], bqk[:])
            bv_t = consts.tile([P, HL * D], F32)
            bpr_t = consts.tile([P, C], F32)
            ones_col = consts.tile([1, D], F32R)

            with (
                tc.tile_pool(name="psum", space="PSUM", bufs=3) as psum,
                tc.tile_pool(name="qk_sb", bufs=2 * PAIRS) as qk_pool,
                tc.tile_pool(name="v_sb", bufs=1) as v_pool,
            ):
                # ---- Phase A1: V = x @ w_v + b_v, laid out [tok, d] per
                # (head, kblock) as [P, 65] slices (col 64 stays 1.0 for the
                # softmax-denominator trick).
                v_sb = v_pool.tile([P, HL * KB * 65], F32R)
                # view [P, head, kb, 65]
                v_view = v_sb[:].rearrange("p (h k c) -> p h k c", h=HL, k=KB)
                bv_view = bv_t[:].rearrange("p (h d) -> p h d", h=HL)

                with (
                    tc.tile_pool(name="wav", bufs=NCHUNK) as wav_pool,
                    tc.tile_pool(name="xtv", bufs=3) as xtv_pool,
                ):
                    wav_sb = [
                        wav_pool.tile([P, HL * D], F32R, tag="wav", name=f"wav{a}")
                        for a in range(NCHUNK)
                    ]
                    nc.sync.dma_start(wav_sb[0][:], wv[0:P, :])
                    xt_r = xt[:].rearrange("(a p) t -> p a t", p=P)
                    xtv_cache = {}

                    def xtv_get(tb):
                        if tb not in xtv_cache:
                            t = xtv_pool.tile(
                                [P, NCHUNK * P], F32R, tag="xtv",
                                name=f"xtv{tb}",
                            )
                            nc.sync.dma_start(
                                t[:].rearrange("p (a t) -> p a t", a=NCHUNK),
                                xt_r[:, :, tb * P : (tb + 1) * P],
                            )
                            xtv_cache[tb] = t
                        return xtv_cache[tb]

                    xtv_get(0)  # first rhs ahead of the remaining weights
                    for a in range(1, NCHUNK):
                        nc.sync.dma_start(
                            wav_sb[a][:], wv[a * P : (a + 1) * P, :]
                        )
                    # deferred non-critical loads: biases, ones column
                    nc.sync.dma_start(bv_t[:], bv[:])
                    nc.sync.dma_start(ones_col[:], onesd[0:1, 0:D])
                    nc.sync.dma_start(
                        v_sb[:].rearrange("p (t c) -> p t c", c=65)[:, :, 64:65],
                        onesd[:].rearrange("p (t c) -> p t c", c=1),
                    )
                    nc.sync.dma_start(bpr_t[:], bpr[:])
                    for tb in range(KB):
                        xtv = xtv_get(tb)
                        v_ps = psum.tile([P, W], F32, tag="mm")
                        for a in range(NCHUNK):
                            nc.tensor.matmul(
                                v_ps[:],
                                _r(xtv[:, a * P : (a + 1) * P]),
                                _r(wav_sb[a][:]),
                                start=(a == 0),
                                stop=(a == NCHUNK - 1),
                            )
                        nc.vector.tensor_add(
                            v_view[:, :, tb, 0:D],
                            v_ps[:].rearrange("p (h d) -> p h d", h=HL),
                            bv_view[:, :, :],
                        )

                # ---- Phase A2: Q_T / K_T = (x @ w_qk + b_qk)^T, laid out
                # [qk-col, tok]; 8 tiles of [128, T], one per head-pair
                # (blocks 0..3 = Q pairs, 4..7 = K pairs).
                qk_sb = []
                for j in range(2 * PAIRS):
                    qk_sb.append(qk_pool.tile([P, T], F32R, tag="qk", name=f"qk{j}"))
                with (
                    tc.tile_pool(name="waqk", bufs=NCHUNK) as waqk_pool,
                    tc.tile_pool(name="xtq", bufs=2) as xtq_pool,
                ):
                    waqk_sb = [
                        waqk_pool.tile([P, C], F32R, tag="waqk", name=f"waqk{a}")
                        for a in range(NCHUNK)
                    ]
                    nc.sync.dma_start(waqk_sb[0][:], wqk[0:P, :])
                    xtq_cache = {}

                    def xtq_get(w):
                        if w not in xtq_cache:
                            t = xtq_pool.tile(
                                [P, NCHUNK * W], F32R, tag="xtq", name=f"xtq{w}"
                            )
                            nc.sync.dma_start(
                                t[:].rearrange("p (a t) -> p a t", a=NCHUNK),
                                xt_r[:, :, w * W : (w + 1) * W],
                            )
                            xtq_cache[w] = t
                        return xtq_cache[w]

                    xtq_get(0)  # first rhs ahead of the remaining weights
                    for a in range(1, NCHUNK):
                        nc.sync.dma_start(
                            waqk_sb[a][:], wqk[a * P : (a + 1) * P, :]
                        )
                    for w in range(NW):
                        xtq = xtq_get(w)
                        for j in range(2 * PAIRS):
                            qk_ps = psum.tile([P, W], F32, tag="mm")
                            for a in range(NCHUNK):
                                nc.tensor.matmul(
                                    qk_ps[:],
                                    _r(waqk_sb[a][:, j * P : (j + 1) * P]),
                                    _r(xtq[:, a * W : (a + 1) * W]),
                                    start=(a == 0),
                                    stop=(a == NCHUNK - 1),
                                )
                            nc.vector.tensor_scalar(
                                out=qk_sb[j][:, w * W : (w + 1) * W],
                                in0=qk_ps[:],
                                scalar1=bqk_t[:, j : j + 1],
                                scalar2=None,
                                op0=mybir.AluOpType.add,
                            )

                # ---- Phase B: causal attention per local head.
                with (
                    tc.tile_pool(name="yt_sb", bufs=PAIRS) as yt_pool,
                    tc.tile_pool(name="attn", bufs=3) as attn_pool,
                    tc.tile_pool(name="norm", bufs=1) as norm_pool,
                ):
                    yt_sb = [yt_pool.tile([P, T], F32R, tag="yt", name=f"yt{i}") for i in range(PAIRS)]
                    for h in range(HL):
                        pr, sub = h // 2, h % 2
                        QT = qk_sb[pr]
                        KT = qk_sb[PAIRS + pr]
                        y_ps = [
                            psum.tile([65, W], F32, tag="y", bufs=4, name=f"y{h}_{i}")
                            for i in range(NW)
                        ]
                        recips = [
                            norm_pool.tile([1, W], F32R, tag="recip", bufs=4, name=f"rc{h}_{i}")
                            for i in range(NW)
                        ]
                        for kb in range(KB):
                            w0 = kb // NW
                            coff = (kb % NW) * P
                            attn_t = attn_pool.tile([P, T], F32R, tag="attn")
                            for w in range(w0, NW):
                                cs = coff if w == w0 else 0
                                s_ps = psum.tile([P, W], F32, tag="mm")
                                nc.tensor.matmul(
                                    s_ps[:, cs:W],
                                    _r(KT[sub * D : sub * D + D, kb * P : (kb + 1) * P]),
                                    _r(QT[sub * D : sub * D + D, w * W + cs : (w + 1) * W]),
                                    start=True,
                                    stop=True,
                                )
                                nc.scalar.activation(
                                    attn_t[:, w * W + cs : (w + 1) * W],
                                    s_ps[:, cs:W],
                                    EXP,
                                    scale=1.0 / np.sqrt(D),
                                )
                                if w == w0:
                                    nc.vector.tensor_mul(
                                        attn_t[:, w0 * W + coff : w0 * W + coff + P],
                                        attn_t[:, w0 * W + coff : w0 * W + coff + P],
                                        tri_t[:],
                                    )
                            for w in range(w0, NW):
                                cs = coff if w == w0 else 0
                                nc.tensor.matmul(
                                    y_ps[w][:, cs:W],
                                    _r(v_sb[:, (h * KB + kb) * 65 : (h * KB + kb) * 65 + 65]),
                                    _r(attn_t[:, w * W + cs : (w + 1) * W]),
                                    start=(kb == 0),
                                    stop=(kb == 4 * w + 3),
                                )
                            if kb % 4 == 3:
                                # window kb//4 is complete: normalize + evict
                                # its PSUM bank while later k-blocks continue.
                                w = kb // 4
                                nc.vector.reciprocal(
                                    recips[w][:], y_ps[w][64:65, :]
                                )
                                bc_ps = psum.tile(
                                    [D, W], F32, tag="bc", bufs=1, name=f"bc{h}_{w}"
                                )
                                nc.tensor.matmul(
                                    bc_ps[:],
                                    _r(ones_col[:]),
                                    _r(recips[w][:]),
                                    start=True,
                                    stop=True,
                                )
                                pbc = norm_pool.tile(
                                    [D, W], F32, tag="pbc", bufs=2, name=f"pbc{h}_{w}"
                                )
                                nc.vector.tensor_copy(pbc[:], bc_ps[:])
                                nc.vector.tensor_mul(
                                    yt_sb[pr][sub * D : sub * D + D, w * W : (w + 1) * W],
                                    y_ps[w][0:D, :],
                                    pbc[:],
                                )

                    # ---- Phase C: partial c_proj: out = y_T.T @ wp (+ bpr).
                    with (
                        tc.tile_pool(name="wp_sb", bufs=PAIRS) as wp_pool,
                        tc.tile_pool(name="osb", bufs=3) as o_pool,
                    ):
                        wp_sb = []
                        for ch in range(PAIRS):
                            t = wp_pool.tile([P, C], F32R, tag="wp", name=f"wp{ch}")
                            nc.sync.dma_start(t[:], wp[ch * P : (ch + 1) * P, :])
                            wp_sb.append(t)
                        for tb in range(KB):
                            for ew in range(C // W):
                                o_ps = psum.tile([P, W], F32, tag="mm")
                                for ch in range(PAIRS):
                                    nc.tensor.matmul(
                                        o_ps[:],
                                        _r(yt_sb[ch][:, tb * P : (tb + 1) * P]),
                                        _r(wp_sb[ch][:, ew * W : (ew + 1) * W]),
                                        start=(ch == 0),
                                        stop=(ch == PAIRS - 1),
                                    )
                                o_sb = o_pool.tile([P, W], F32, tag="osb")
                                nc.vector.tensor_add(
                                    o_sb[:], o_ps[:], bpr_t[:, ew * W : (ew + 1) * W]
                                )
                                nc.sync.dma_start(
                                    out[tb * P : (tb + 1) * P, ew * W : (ew + 1) * W],
                                    o_sb[:],
                                )

    nc.compile()
    _CACHE["nc"] = nc
    return nc


def make_in_maps(x, w_attn, b_attn, w_proj, b_proj):
    """Host-side sharding: per-core input dict."""
    x = np.ascontiguousarray(np.asarray(x, dtype=np.float32))
    w_attn = np.asarray(w_attn, dtype=np.float32)
    b_attn = np.asarray(b_attn, dtype=np.float32)
    w_proj = np.asarray(w_proj, dtype=np.float32)
    b_proj = np.asarray(b_proj, dtype=np.float32)

    trimask = np.triu(np.ones((P, P), dtype=np.float32))  # [k, q]: 1 if q >= k
    in_maps = []
    for c in range(N_CORES):
        b = c // 2
        g = c % 2
        h0 = g * HL
        # Q/K columns arranged pair-wise: [q(h0) q(h0+1) | q(h0+2) ... | k(...)]
        qcols = np.arange(h0 * D, (h0 + HL) * D)
        kcols = C + qcols
        wqk = np.concatenate(
            [w_attn[:, qcols], w_attn[:, kcols]], axis=1
        )  # [C, 1024]
        bqk_flat = np.concatenate([b_attn[qcols], b_attn[kcols]])  # [1024]
        bqk = np.ascontiguousarray(bqk_flat.reshape(2 * PAIRS, P).T)  # [128, 8]
        vcols = 2 * C + np.arange(h0 * D, (h0 + HL) * D)
        wv = np.ascontiguousarray(w_attn[:, vcols])  # [C, 512]
        bv = np.broadcast_to(b_attn[vcols], (P, HL * D)).copy()
        wp = np.ascontiguousarray(w_proj[h0 * D : (h0 + HL) * D, :])  # [512, C]
        if g == 0:
            bpr = np.broadcast_to(b_proj, (P, C)).copy()
        else:
            bpr = np.zeros((P, C), dtype=np.float32)
        in_maps.append(
            {
                "xt": np.ascontiguousarray(x[b].T),  # [C, T]
                "wqk": wqk,
                "wv": wv,
                "bqk": bqk,
                "bv": bv,
                "wp": wp,
                "bpr": bpr,
                "trimask": trimask,
                "onesd": np.ones((P, P), dtype=np.float32),
            }
        )
    return in_maps


def kernel(x, w_attn, b_attn, w_proj, b_proj, _trace=False):
    global LAST_RESULTS
    nc = build_nc()
    in_maps = make_in_maps(x, w_attn, b_attn, w_proj, b_proj)
    res = run_bass_kernel_spmd(
        nc, in_maps, list(range(N_CORES)), trace=_trace
    )
    LAST_RESULTS = res
    outs = [res.results[c]["out"] for c in range(N_CORES)]
    y = np.stack([outs[2 * b] + outs[2 * b + 1] for b in range(B)], axis=0)
    return y.astype(np.float32)



# revision 7
# speedup vs baseline: 1.0921x; 1.0921x over previous
"""Causal self-attention (B=4, T=2048, C=1024, H=16) on 8 TRN2 NeuronCores.

Sharding: hybrid batch x head tensor-parallel. Core c handles batch b = c//2
and heads [8*(c%2) : 8*(c%2)+8]. Each core computes QKV for its 8 heads over
its batch, full causal attention for those heads, and a *partial* c_proj
(contribution of its 8 heads to all 2048 tokens of its batch). The host
unshards by summing the two partial outputs of each batch pair; b_proj is
added on-device by the even core of each pair.

Single-pass pipelined structure (per core): one loop over the 4 q-windows of
512 tokens. Per window w: QK projections for that token window (Q kept only
for the window, K appended to a persistent K_T), V for the window's 4 token
blocks, then causal attention for all 8 heads over k-blocks 0..4w+3 (scores
matmul -> exp on Act -> triangular mask on DVE -> attn@V accumulate), per-head
softmax normalization (denominator rides in PSUM row 64 via a ones column in
the V tiles), then the window's partial c_proj with the bias folded in as a
rank-1 accumulate matmul and gpsimd moving PSUM->SBUF for the output DMA.
x is fed transposed (xt [C, T]) and loaded once per window; weights stay
resident in SBUF. All matmuls are float32r (full-rate fp32, moving dim kept
>= 256 everywhere: the 128-wide diagonal chunks are widened to 256 with a
zero-extended triangular mask).
"""

import numpy as np

import concourse.bass as bass
import concourse.mybir as mybir
import concourse.tile as tile
from concourse import bacc
from concourse.bass_utils import run_bass_kernel_spmd

B, T, C = 4, 2048, 1024
H = 16          # total heads
HL = 8          # heads per core
D = 64          # head dim
P = 128
W = 512         # q-window / matmul moving-dim size
NW = T // W     # 4 q windows
KB = T // P     # 16 k blocks
NCHUNK = C // P  # 8 contraction chunks over C
PAIRS = HL // 2  # 4 head-pairs (2 heads per 128-partition tile)
F32 = mybir.dt.float32
F32R = mybir.dt.float32r
EXP = mybir.ActivationFunctionType.Exp
N_CORES = 8
LAG = 3          # scores->attn@V software pipeline depth per head

_CACHE = {}
LAST_RESULTS = None


def build_nc():
    if "nc" in _CACHE:
        return _CACHE["nc"]
    nc = bacc.Bacc(
        "TRN2", target_bir_lowering=False, debug=False, num_devices=N_CORES
    )

    xt = nc.dram_tensor("xt", [C, T], F32R, kind="ExternalInput")
    wqk = nc.dram_tensor("wqk", [C, C], F32R, kind="ExternalInput")
    wv = nc.dram_tensor("wv", [C, HL * D], F32R, kind="ExternalInput")
    bqk = nc.dram_tensor("bqk", [P, 2 * PAIRS], F32, kind="ExternalInput")
    bv = nc.dram_tensor("bv", [P, HL * D], F32, kind="ExternalInput")
    wp = nc.dram_tensor("wp", [HL * D, C], F32R, kind="ExternalInput")
    bpr = nc.dram_tensor("bpr", [1, C], F32R, kind="ExternalInput")
    trimask = nc.dram_tensor("trimask", [P, P], F32, kind="ExternalInput")
    trimask2 = nc.dram_tensor("trimask2", [P, 2 * P], F32, kind="ExternalInput")
    onesd = nc.dram_tensor("onesd", [P, P], F32R, kind="ExternalInput")
    out = nc.dram_tensor("out", [T, C], F32, kind="ExternalOutput")

    xt_r = xt[:].rearrange("(a p) t -> p a t", p=P)

    with tile.TileContext(nc) as tc, nc.allow_low_precision(
        reason="float32r tiles for full-rate fp32 PE matmuls"
    ):
        with (
            tc.tile_pool(name="consts", bufs=1) as consts,
            tc.tile_pool(name="waqk", bufs=NCHUNK) as waqk_pool,
            tc.tile_pool(name="wav", bufs=NCHUNK) as wav_pool,
            tc.tile_pool(name="xtw", bufs=1) as xtw_pool,
            tc.tile_pool(name="kt", bufs=1) as kt_pool,
            tc.tile_pool(name="qt", bufs=PAIRS) as qt_pool,
            tc.tile_pool(name="vsb", bufs=1) as v_pool,
            tc.tile_pool(name="attn", bufs=LAG + 1) as attn_pool,
            tc.tile_pool(name="yt", bufs=1) as yt_pool,
            tc.tile_pool(name="wp_sb", bufs=1) as wp_pool,
            tc.tile_pool(name="osb", bufs=3) as o_pool,
            tc.tile_pool(name="norm", bufs=2) as norm_pool,
            tc.tile_pool(name="psum", space="PSUM", bufs=4) as psum,
        ):
            # ---- const tiles
            bqk_t = consts.tile([P, 2 * PAIRS], F32)
            bv_t = consts.tile([P, HL * D], F32)
            tri_t = consts.tile([P, P], F32)
            tri2_t = consts.tile([P, 2 * P], F32)
            ones_row = consts.tile([1, P], F32R)
            bpr_t = consts.tile([1, C], F32R)

            waqk_sb = [
                waqk_pool.tile([P, C], F32R, tag="waqk", name=f"waqk{a}")
                for a in range(NCHUNK)
            ]
            wav_sb = [
                wav_pool.tile([P, HL * D], F32R, tag="wav", name=f"wav{a}")
                for a in range(NCHUNK)
            ]
            kt_sb = [
                kt_pool.tile([P, T], F32R, tag=f"kt{pr}", name=f"kt{pr}")
                for pr in range(PAIRS)
            ]
            wp_sb = [
                wp_pool.tile([P, C], F32R, tag=f"wp{ch}", name=f"wp{ch}")
                for ch in range(PAIRS)
            ]
            # V laid out [tok, d] per (head, kblock) as [P, 65] slices
            # (col 64 stays 1.0 so attn@V accumulates softmax denominators).
            v_sb = v_pool.tile([P, HL * KB * 65], F32R)
            v_view = v_sb[:].rearrange("p (h k c) -> p h k c", h=HL, k=KB)
            bv_view = bv_t[:].rearrange("p (h d) -> p h d", h=HL)

            # ---- DMA kickoff, window-0 critical path first: Q-half weight
            # chunks interleaved with xt window-0 chunks, then K halves,
            # then wv; everything else after.
            xtw_tiles = {}

            def xtw_get(w):
                if w not in xtw_tiles:
                    t = xtw_pool.tile([P, NCHUNK * W], F32R, tag="xtw",
                                      name=f"xtw{w}")
                    tv = t[:].rearrange("p (a t) -> p a t", a=NCHUNK)
                    for a in range(NCHUNK):
                        nc.sync.dma_start(
                            tv[:, a, :], xt_r[:, a, w * W : (w + 1) * W]
                        )
                    xtw_tiles[w] = t
                return xtw_tiles[w]

            for a in range(NCHUNK):
                nc.sync.dma_start(
                    waqk_sb[a][:, 0:W], wqk[a * P : (a + 1) * P, 0:W]
                )
                if a == 0:
                    nc.sync.dma_start(bqk_t[:], bqk[:])
                    xtw_get(0)
            for a in range(NCHUNK):
                nc.sync.dma_start(
                    waqk_sb[a][:, W:C], wqk[a * P : (a + 1) * P, W:C]
                )
            for a in range(NCHUNK):
                nc.sync.dma_start(wav_sb[a][:], wv[a * P : (a + 1) * P, :])
            nc.sync.dma_start(bv_t[:], bv[:])
            nc.sync.dma_start(
                v_sb[:].rearrange("p (t c) -> p t c", c=65)[:, :, 64:65],
                onesd[:].rearrange("p (t c) -> p t c", c=1),
            )
            nc.sync.dma_start(ones_row[:], onesd[0:1, :])
            nc.sync.dma_start(tri_t[:], trimask[:])
            nc.sync.dma_start(tri2_t[:], trimask2[:])
            for ch in range(PAIRS):
                nc.sync.dma_start(wp_sb[ch][:], wp[ch * P : (ch + 1) * P, :])
            nc.sync.dma_start(bpr_t[:], bpr[:])

            qt_sb = [None] * PAIRS

            def emit_qk_copy(j, qk_ps, w):
                # move PSUM -> SBUF with the per-qk-column bias added
                if j < PAIRS:
                    qt_sb[j] = qt_pool.tile(
                        [P, W], F32R, tag=f"qt{j}", bufs=1, name=f"qt{j}_{w}"
                    )
                    dest = qt_sb[j][:]
                else:
                    dest = kt_sb[j - PAIRS][:, w * W : (w + 1) * W]
                nc.vector.tensor_scalar(
                    out=dest,
                    in0=qk_ps[:],
                    scalar1=bqk_t[:, j : j + 1],
                    scalar2=None,
                    op0=mybir.AluOpType.add,
                )

            def emit_v_add(i, v_ps, w):
                tb = 4 * w + i
                nc.vector.tensor_add(
                    v_view[:, :, tb, 0:D],
                    v_ps[:].rearrange("p (h d) -> p h d", h=HL),
                    bv_view[:, :, :],
                )

            def emit_qkv_window0():
                xtw = xtw_get(0)
                # chunk-major over 4-tile groups so PE can trail the DMA
                # stream chunk by chunk.
                for jg in range(2):  # Q pairs then K pairs
                    qk_ps = [
                        psum.tile([P, W], F32, tag="mm", name=f"qk0_{jg}{j}")
                        for j in range(4)
                    ]
                    for a in range(NCHUNK):
                        for j in range(4):
                            nc.tensor.matmul(
                                qk_ps[j][:],
                                waqk_sb[a][:, (4 * jg + j) * P : (4 * jg + j + 1) * P],
                                xtw[:, a * W : (a + 1) * W],
                                start=(a == 0),
                                stop=(a == NCHUNK - 1),
                            )
                    for j in range(4):
                        emit_qk_copy(4 * jg + j, qk_ps[j], 0)
                v_ps = [
                    psum.tile([P, W], F32, tag="mm", name=f"v0_{i}")
                    for i in range(4)
                ]
                for a in range(NCHUNK):
                    for i in range(4):
                        nc.tensor.matmul(
                            v_ps[i][:],
                            xtw[:, a * W + i * P : a * W + (i + 1) * P],
                            wav_sb[a][:],
                            start=(a == 0),
                            stop=(a == NCHUNK - 1),
                        )
                for i in range(4):
                    emit_v_add(i, v_ps[i], 0)

            def emit_qkv_window(w):
                # windows >= 1: inputs already resident, j-major streaming
                xtw = xtw_get(w)
                for j in range(2 * PAIRS):
                    qk_ps = psum.tile([P, W], F32, tag="mm", name=f"qk{w}_{j}")
                    for a in range(NCHUNK):
                        nc.tensor.matmul(
                            qk_ps[:],
                            waqk_sb[a][:, j * P : (j + 1) * P],
                            xtw[:, a * W : (a + 1) * W],
                            start=(a == 0),
                            stop=(a == NCHUNK - 1),
                        )
                    emit_qk_copy(j, qk_ps, w)
                for i in range(4):
                    v_ps = psum.tile([P, W], F32, tag="mm", name=f"v{w}_{i}")
                    for a in range(NCHUNK):
                        nc.tensor.matmul(
                            v_ps[:],
                            xtw[:, a * W + i * P : a * W + (i + 1) * P],
                            wav_sb[a][:],
                            start=(a == 0),
                            stop=(a == NCHUNK - 1),
                        )
                    emit_v_add(i, v_ps, w)

            emit_qkv_window0()

            for w in range(NW):
                nkb = 4 * w + 4
                yt_w = [
                    yt_pool.tile([P, W], F32R, tag=f"yt{pr}", bufs=1,
                                 name=f"yt{pr}_{w}")
                    for pr in range(PAIRS)
                ]
                for h in range(HL):
                    pr, sub = h // 2, h % 2
                    QT = qt_sb[pr]
                    KT = kt_sb[pr]
                    y_ps = psum.tile([65, W], F32, tag="y", bufs=2,
                                     name=f"y{w}_{h}")
                    pending = []

                    def emit_scores(kb):
                        if kb < 4 * w:
                            cs, mk = 0, None
                        else:
                            i = kb - 4 * w
                            cs = (0, P, 2 * P, 2 * P)[i]
                            mk = i
                        s_ps = psum.tile([P, W], F32, tag="mm",
                                         name=f"s{w}_{h}_{kb}")
                        at = attn_pool.tile([P, W], F32R, tag="attn")
                        nc.tensor.matmul(
                            s_ps[:, cs:W],
                            KT[sub * D : (sub + 1) * D, kb * P : (kb + 1) * P],
                            QT[sub * D : (sub + 1) * D, cs:W],
                            start=True,
                            stop=True,
                        )
                        nc.scalar.activation(
                            at[:, cs:W], s_ps[:, cs:W], EXP,
                            scale=1.0 / np.sqrt(D),
                        )
                        if mk is not None:
                            if mk < 3:
                                nc.vector.tensor_mul(
                                    at[:, mk * P : (mk + 1) * P],
                                    at[:, mk * P : (mk + 1) * P],
                                    tri_t[:],
                                )
                            else:
                                nc.vector.tensor_mul(
                                    at[:, 2 * P : W],
                                    at[:, 2 * P : W],
                                    tri2_t[:],
                                )
                        return (kb, cs, at)

                    def emit_av(kb, cs, at):
                        nc.tensor.matmul(
                            y_ps[:, cs:W],
                            v_sb[:, (h * KB + kb) * 65 : (h * KB + kb + 1) * 65],
                            at[:, cs:W],
                            start=(kb == 0),
                            stop=(kb == nkb - 1),
                        )

                    for kb in range(nkb):
                        pending.append(emit_scores(kb))
                        if len(pending) > LAG:
                            emit_av(*pending.pop(0))
                    for item in pending:
                        emit_av(*item)

                    # softmax normalization: divide y rows by the denominator
                    # accumulated in PSUM row 64.
                    rc = norm_pool.tile([1, W], F32R, tag="recip",
                                        name=f"rc{w}_{h}")
                    nc.vector.reciprocal(rc[:], y_ps[64:65, :])
                    bc_ps = psum.tile([D, W], F32, tag="bc", bufs=2,
                                      name=f"bc{w}_{h}")
                    nc.tensor.matmul(
                        bc_ps[:], ones_row[0:1, 0:D], rc[:],
                        start=True, stop=True,
                    )
                    pbc = norm_pool.tile([D, W], F32, tag="pbc",
                                         name=f"pbc{w}_{h}")
                    nc.vector.tensor_copy(pbc[:], bc_ps[:])
                    nc.vector.tensor_mul(
                        yt_w[pr][sub * D : (sub + 1) * D, :],
                        y_ps[0:D, :],
                        pbc[:],
                    )

                # QKV for the next window before this window's c_proj so PE
                # has independent work while the last head's normalization
                # drains, and so Act/DVE catch up before c_proj needs them.
                if w + 1 < NW:
                    xtw_get(w + 1)
                    emit_qkv_window(w + 1)

                # partial c_proj for this window's 4 token blocks; bias goes
                # in as a rank-1 accumulate, gpsimd moves PSUM -> SBUF.
                for i in range(4):
                    tb = 4 * w + i
                    for ew in range(C // W):
                        o_ps = psum.tile([P, W], F32, tag="mm",
                                         name=f"o{tb}_{ew}")
                        for ch in range(PAIRS):
                            nc.tensor.matmul(
                                o_ps[:],
                                yt_w[ch][:, i * P : (i + 1) * P],
                                wp_sb[ch][:, ew * W : (ew + 1) * W],
                                start=(ch == 0),
                                stop=False,
                            )
                        nc.tensor.matmul(
                            o_ps[:],
                            ones_row[0:1, 0:P],
                            bpr_t[0:1, ew * W : (ew + 1) * W],
                            start=False,
                            stop=True,
                        )
                        o_sb = o_pool.tile([P, W], F32, tag="osb")
                        nc.scalar.copy(o_sb[:], o_ps[:])
                        nc.sync.dma_start(
                            out[tb * P : (tb + 1) * P, ew * W : (ew + 1) * W],
                            o_sb[:],
                        )

    nc.compile()
    _CACHE["nc"] = nc
    return nc


def make_in_maps(x, w_attn, b_attn, w_proj, b_proj):
    """Host-side sharding: per-core input dict."""
    x = np.ascontiguousarray(np.asarray(x, dtype=np.float32))
    w_attn = np.asarray(w_attn, dtype=np.float32)
    b_attn = np.asarray(b_attn, dtype=np.float32)
    w_proj = np.asarray(w_proj, dtype=np.float32)
    b_proj = np.asarray(b_proj, dtype=np.float32)

    trimask = np.triu(np.ones((P, P), dtype=np.float32))  # [k, q]: 1 if q >= k
    trimask2 = np.concatenate(
        [np.zeros((P, P), dtype=np.float32), trimask], axis=1
    )
    in_maps = []
    for c in range(N_CORES):
        b = c // 2
        g = c % 2
        h0 = g * HL
        # Q/K columns arranged pair-wise: [q(h0) q(h0+1) | q(h0+2) ... | k(...)]
        qcols = np.arange(h0 * D, (h0 + HL) * D)
        kcols = C + qcols
        wqk = np.concatenate(
            [w_attn[:, qcols], w_attn[:, kcols]], axis=1
        )  # [C, 1024]
        bqk_flat = np.concatenate([b_attn[qcols], b_attn[kcols]])  # [1024]
        bqk = np.ascontiguousarray(bqk_flat.reshape(2 * PAIRS, P).T)  # [128, 8]
        vcols = 2 * C + np.arange(h0 * D, (h0 + HL) * D)
        wv = np.ascontiguousarray(w_attn[:, vcols])  # [C, 512]
        bv = np.broadcast_to(b_attn[vcols], (P, HL * D)).copy()
        wp = np.ascontiguousarray(w_proj[h0 * D : (h0 + HL) * D, :])  # [512, C]
        if g == 0:
            bpr = b_proj.reshape(1, C).copy()
        else:
            bpr = np.zeros((1, C), dtype=np.float32)
        in_maps.append(
            {
                "xt": np.ascontiguousarray(x[b].T),  # [C, T]
                "wqk": wqk,
                "wv": wv,
                "bqk": bqk,
                "bv": bv,
                "wp": wp,
                "bpr": bpr,
                "trimask": trimask,
                "trimask2": trimask2,
                "onesd": np.ones((P, P), dtype=np.float32),
            }
        )
    return in_maps


def kernel(x, w_attn, b_attn, w_proj, b_proj, _trace=False):
    global LAST_RESULTS
    nc = build_nc()
    in_maps = make_in_maps(x, w_attn, b_attn, w_proj, b_proj)
    res = run_bass_kernel_spmd(
        nc, in_maps, list(range(N_CORES)), trace=_trace
    )
    LAST_RESULTS = res
    outs = [res.results[c]["out"] for c in range(N_CORES)]
    y = np.stack([outs[2 * b] + outs[2 * b + 1] for b in range(B)], axis=0)
    return y.astype(np.float32)


# revision 9
# speedup vs baseline: 1.1095x; 1.0160x over previous
"""Causal self-attention (B=4, T=2048, C=1024, H=16) on 8 TRN2 NeuronCores.

Sharding: hybrid batch x head tensor-parallel. Core c handles batch b = c//2
and heads [8*(c%2) : 8*(c%2)+8]. Each core computes QKV for its 8 heads over
its batch, full causal attention for those heads, and a *partial* c_proj
(contribution of its 8 heads to all 2048 tokens of its batch). The host
unshards by summing the two partial outputs of each batch pair; b_proj is
added on-device by the even core of each pair.

Single-pass pipelined structure (per core): one loop over the 4 q-windows of
512 tokens. Per window w: QK projections for that token window (Q kept only
for the window, K appended to a persistent K_T), V for the window's 4 token
blocks, then causal attention for all 8 heads over k-blocks 0..4w+3 (scores
matmul -> exp on Act -> triangular mask on DVE -> attn@V accumulate), per-head
softmax normalization (denominator rides in PSUM row 64 via a ones column in
the V tiles), then the window's partial c_proj with the bias folded in as a
rank-1 accumulate matmul and gpsimd moving PSUM->SBUF for the output DMA.
x is fed transposed (xt [C, T]) and loaded once per window; weights stay
resident in SBUF. All matmuls are float32r (full-rate fp32, moving dim kept
>= 256 everywhere: the 128-wide diagonal chunks are widened to 256 with a
zero-extended triangular mask).
"""

import numpy as np

import concourse.bass as bass
import concourse.mybir as mybir
import concourse.tile as tile
from concourse import bacc
from concourse.bass_utils import run_bass_kernel_spmd

B, T, C = 4, 2048, 1024
H = 16          # total heads
HL = 8          # heads per core
D = 64          # head dim
P = 128
W = 512         # q-window / matmul moving-dim size
NW = T // W     # 4 q windows
KB = T // P     # 16 k blocks
NCHUNK = C // P  # 8 contraction chunks over C
PAIRS = HL // 2  # 4 head-pairs (2 heads per 128-partition tile)
F32 = mybir.dt.float32
F32R = mybir.dt.float32r
EXP = mybir.ActivationFunctionType.Exp
N_CORES = 8
LAG = 3          # scores->attn@V software pipeline depth per head

_CACHE = {}
LAST_RESULTS = None


def build_nc():
    if "nc" in _CACHE:
        return _CACHE["nc"]
    nc = bacc.Bacc(
        "TRN2", target_bir_lowering=False, debug=False, num_devices=N_CORES
    )

    xt = nc.dram_tensor("xt", [C, T], F32R, kind="ExternalInput")
    wqk = nc.dram_tensor("wqk", [C, C], F32R, kind="ExternalInput")
    wv = nc.dram_tensor("wv", [C, HL * D], F32R, kind="ExternalInput")
    bqk = nc.dram_tensor("bqk", [P, 2 * PAIRS], F32, kind="ExternalInput")
    bv = nc.dram_tensor("bv", [P, HL * D], F32, kind="ExternalInput")
    wp = nc.dram_tensor("wp", [HL * D, C], F32R, kind="ExternalInput")
    bpr = nc.dram_tensor("bpr", [P, C], F32, kind="ExternalInput")
    trimask = nc.dram_tensor("trimask", [P, P], F32, kind="ExternalInput")
    trimask2 = nc.dram_tensor("trimask2", [P, 2 * P], F32, kind="ExternalInput")
    onesd = nc.dram_tensor("onesd", [P, P], F32R, kind="ExternalInput")
    out = nc.dram_tensor("out", [T, C], F32, kind="ExternalOutput")

    xt_r = xt[:].rearrange("(a p) t -> p a t", p=P)

    with tile.TileContext(nc) as tc, nc.allow_low_precision(
        reason="float32r tiles for full-rate fp32 PE matmuls"
    ):
        with (
            tc.tile_pool(name="consts", bufs=1) as consts,
            tc.tile_pool(name="waqk", bufs=NCHUNK) as waqk_pool,
            tc.tile_pool(name="wav", bufs=NCHUNK) as wav_pool,
            tc.tile_pool(name="xtw", bufs=1) as xtw_pool,
            tc.tile_pool(name="kt", bufs=1) as kt_pool,
            tc.tile_pool(name="qt", bufs=PAIRS) as qt_pool,
            tc.tile_pool(name="vsb", bufs=1) as v_pool,
            tc.tile_pool(name="attn", bufs=LAG + 1) as attn_pool,
            tc.tile_pool(name="yt", bufs=1) as yt_pool,
            tc.tile_pool(name="wp_sb", bufs=1) as wp_pool,
            tc.tile_pool(name="osb", bufs=3) as o_pool,
            tc.tile_pool(name="norm", bufs=2) as norm_pool,
            tc.tile_pool(name="psum", space="PSUM", bufs=4) as psum,
        ):
            # ---- const tiles
            bqk_t = consts.tile([P, 2 * PAIRS], F32)
            bv_t = consts.tile([P, HL * D], F32)
            tri_t = consts.tile([P, P], F32)
            tri2_t = consts.tile([P, 2 * P], F32)
            ones_row = consts.tile([1, P], F32R)
            bpr_t = consts.tile([P, C], F32)

            waqk_sb = [
                waqk_pool.tile([P, C], F32R, tag="waqk", name=f"waqk{a}")
                for a in range(NCHUNK)
            ]
            wav_sb = [
                wav_pool.tile([P, HL * D], F32R, tag="wav", name=f"wav{a}")
                for a in range(NCHUNK)
            ]
            kt_sb = [
                kt_pool.tile([P, T], F32R, tag=f"kt{pr}", name=f"kt{pr}")
                for pr in range(PAIRS)
            ]
            wp_sb = [
                wp_pool.tile([P, C], F32R, tag=f"wp{ch}", name=f"wp{ch}")
                for ch in range(PAIRS)
            ]
            # V laid out [tok, d] per (head, kblock) as [P, 65] slices
            # (col 64 stays 1.0 so attn@V accumulates softmax denominators).
            v_sb = v_pool.tile([P, HL * KB * 65], F32R)
            v_view = v_sb[:].rearrange("p (h k c) -> p h k c", h=HL, k=KB)
            bv_view = bv_t[:].rearrange("p (h d) -> p h d", h=HL)

            # ---- DMA kickoff, window-0 critical path first: Q-half weight
            # chunks interleaved with xt window-0 chunks, then K halves,
            # then wv; everything else after.
            xtw_tiles = {}

            def xtw_get(w):
                if w not in xtw_tiles:
                    t = xtw_pool.tile([P, NCHUNK * W], F32R, tag="xtw",
                                      name=f"xtw{w}")
                    tv = t[:].rearrange("p (a t) -> p a t", a=NCHUNK)
                    for a in range(NCHUNK):
                        nc.sync.dma_start(
                            tv[:, a, :], xt_r[:, a, w * W : (w + 1) * W]
                        )
                    xtw_tiles[w] = t
                return xtw_tiles[w]

            for a in range(NCHUNK):
                nc.sync.dma_start(
                    waqk_sb[a][:, 0:W], wqk[a * P : (a + 1) * P, 0:W]
                )
                if a == 0:
                    nc.sync.dma_start(bqk_t[:], bqk[:])
                    xtw_get(0)
            for a in range(NCHUNK):
                nc.sync.dma_start(
                    waqk_sb[a][:, W:C], wqk[a * P : (a + 1) * P, W:C]
                )
            for a in range(NCHUNK):
                nc.sync.dma_start(wav_sb[a][:], wv[a * P : (a + 1) * P, :])
            nc.sync.dma_start(bv_t[:], bv[:])
            nc.sync.dma_start(
                v_sb[:].rearrange("p (t c) -> p t c", c=65)[:, :, 64:65],
                onesd[:].rearrange("p (t c) -> p t c", c=1),
            )
            nc.sync.dma_start(ones_row[:], onesd[0:1, :])
            nc.sync.dma_start(tri_t[:], trimask[:])
            nc.sync.dma_start(tri2_t[:], trimask2[:])
            for ch in range(PAIRS):
                nc.sync.dma_start(wp_sb[ch][:], wp[ch * P : (ch + 1) * P, :])
            nc.sync.dma_start(bpr_t[:], bpr[:])

            qt_sb = [None] * PAIRS

            def emit_qk_copy(j, qk_ps, w):
                # move PSUM -> SBUF with the per-qk-column bias added
                if j < PAIRS:
                    qt_sb[j] = qt_pool.tile(
                        [P, W], F32R, tag=f"qt{j}", bufs=1, name=f"qt{j}_{w}"
                    )
                    dest = qt_sb[j][:]
                else:
                    dest = kt_sb[j - PAIRS][:, w * W : (w + 1) * W]
                nc.vector.tensor_scalar(
                    out=dest,
                    in0=qk_ps[:],
                    scalar1=bqk_t[:, j : j + 1],
                    scalar2=None,
                    op0=mybir.AluOpType.add,
                )

            def emit_v_add(i, v_ps, w):
                tb = 4 * w + i
                nc.vector.tensor_add(
                    v_view[:, :, tb, 0:D],
                    v_ps[:].rearrange("p (h d) -> p h d", h=HL),
                    bv_view[:, :, :],
                )

            def emit_qkv_window0():
                xtw = xtw_get(0)
                # chunk-major over 4-tile groups so PE can trail the DMA
                # stream chunk by chunk.
                for jg in range(2):  # Q pairs then K pairs
                    qk_ps = [
                        psum.tile([P, W], F32, tag="mm", name=f"qk0_{jg}{j}")
                        for j in range(4)
                    ]
                    for a in range(NCHUNK):
                        for j in range(4):
                            nc.tensor.matmul(
                                qk_ps[j][:],
                                waqk_sb[a][:, (4 * jg + j) * P : (4 * jg + j + 1) * P],
                                xtw[:, a * W : (a + 1) * W],
                                start=(a == 0),
                                stop=(a == NCHUNK - 1),
                            )
                    for j in range(4):
                        emit_qk_copy(4 * jg + j, qk_ps[j], 0)
                v_ps = [
                    psum.tile([P, W], F32, tag="mm", name=f"v0_{i}")
                    for i in range(4)
                ]
                for a in range(NCHUNK):
                    for i in range(4):
                        nc.tensor.matmul(
                            v_ps[i][:],
                            xtw[:, a * W + i * P : a * W + (i + 1) * P],
                            wav_sb[a][:],
                            start=(a == 0),
                            stop=(a == NCHUNK - 1),
                        )
                for i in range(4):
                    emit_v_add(i, v_ps[i], 0)

            def qkv_window_units(w):
                # windows >= 1: inputs already resident, j-major streaming.
                # Returns one closure per projection unit so the caller can
                # interleave them between attention heads as PE filler.
                xtw = xtw_get(w)

                def qk_unit(j):
                    def emit():
                        qk_ps = psum.tile([P, W], F32, tag="mm",
                                          name=f"qk{w}_{j}")
                        for a in range(NCHUNK):
                            nc.tensor.matmul(
                                qk_ps[:],
                                waqk_sb[a][:, j * P : (j + 1) * P],
                                xtw[:, a * W : (a + 1) * W],
                                start=(a == 0),
                                stop=(a == NCHUNK - 1),
                            )
                        emit_qk_copy(j, qk_ps, w)
                    return emit

                def v_unit(i):
                    def emit():
                        v_ps = psum.tile([P, W], F32, tag="mm",
                                         name=f"v{w}_{i}")
                        for a in range(NCHUNK):
                            nc.tensor.matmul(
                                v_ps[:],
                                xtw[:, a * W + i * P : a * W + (i + 1) * P],
                                wav_sb[a][:],
                                start=(a == 0),
                                stop=(a == NCHUNK - 1),
                            )
                        emit_v_add(i, v_ps, w)
                    return emit

                # per-head filler schedule: pair p's Q tile (bufs=1) is
                # only dead after head 2p+1 of the current window, so its
                # qk units may not be emitted earlier; V slots are disjoint.
                return {
                    0: [],
                    1: [qk_unit(0), qk_unit(4)],
                    2: [v_unit(0)],
                    3: [qk_unit(1), qk_unit(5)],
                    4: [v_unit(1)],
                    5: [qk_unit(2), qk_unit(6)],
                    6: [v_unit(2)],
                    7: [qk_unit(3), qk_unit(7), v_unit(3)],
                }

            emit_qkv_window0()

            for w in range(NW):
                nkb = 4 * w + 4
                filler = qkv_window_units(w + 1) if w + 1 < NW else {}
                yt_w = [
                    yt_pool.tile([P, W], F32R, tag=f"yt{pr}", bufs=1,
                                 name=f"yt{pr}_{w}")
                    for pr in range(PAIRS)
                ]
                for h in range(HL):
                    pr, sub = h // 2, h % 2
                    QT = qt_sb[pr]
                    KT = kt_sb[pr]
                    y_ps = psum.tile([65, W], F32, tag="y", bufs=2,
                                     name=f"y{w}_{h}")
                    pending = []

                    def emit_scores(kb):
                        if kb < 4 * w:
                            cs, mk = 0, None
                        else:
                            i = kb - 4 * w
                            cs = (0, P, 2 * P, 2 * P)[i]
                            mk = i
                        s_ps = psum.tile([P, W], F32, tag="mm",
                                         name=f"s{w}_{h}_{kb}")
                        at = attn_pool.tile([P, W], F32R, tag="attn")
                        nc.tensor.matmul(
                            s_ps[:, cs:W],
                            KT[sub * D : (sub + 1) * D, kb * P : (kb + 1) * P],
                            QT[sub * D : (sub + 1) * D, cs:W],
                            start=True,
                            stop=True,
                        )
                        nc.scalar.activation(
                            at[:, cs:W], s_ps[:, cs:W], EXP,
                            scale=1.0 / np.sqrt(D),
                        )
                        if mk is not None:
                            if mk < 3:
                                nc.vector.tensor_mul(
                                    at[:, mk * P : (mk + 1) * P],
                                    at[:, mk * P : (mk + 1) * P],
                                    tri_t[:],
                                )
                            else:
                                nc.vector.tensor_mul(
                                    at[:, 2 * P : W],
                                    at[:, 2 * P : W],
                                    tri2_t[:],
                                )
                        return (kb, cs, at)

                    def emit_av(kb, cs, at):
                        nc.tensor.matmul(
                            y_ps[:, cs:W],
                            v_sb[:, (h * KB + kb) * 65 : (h * KB + kb + 1) * 65],
                            at[:, cs:W],
                            start=(kb == 0),
                            stop=(kb == nkb - 1),
                        )

                    for kb in range(nkb):
                        pending.append(emit_scores(kb))
                        if len(pending) > LAG:
                            emit_av(*pending.pop(0))
                    for item in pending:
                        emit_av(*item)

                    # softmax normalization: divide y rows by the denominator
                    # accumulated in PSUM row 64.
                    rc = norm_pool.tile([1, W], F32R, tag="recip",
                                        name=f"rc{w}_{h}")
                    nc.vector.reciprocal(rc[:], y_ps[64:65, :])
                    bc_ps = psum.tile([D, W], F32, tag="bc", bufs=2,
                                      name=f"bc{w}_{h}")
                    nc.tensor.matmul(
                        bc_ps[:], ones_row[0:1, 0:D], rc[:],
                        start=True, stop=True,
                    )
                    pbc = norm_pool.tile([D, W], F32, tag="pbc",
                                         name=f"pbc{w}_{h}")
                    nc.vector.tensor_copy(pbc[:], bc_ps[:])
                    nc.vector.tensor_mul(
                        yt_w[pr][sub * D : (sub + 1) * D, :],
                        y_ps[0:D, :],
                        pbc[:],
                    )

                    # next-window QKV units as PE filler while Act catches
                    # up on this window's exp backlog
                    for unit in filler.get(h, []):
                        unit()

                # partial c_proj for this window's 4 token blocks; bias goes
                # in as a rank-1 accumulate, gpsimd moves PSUM -> SBUF.
                for i in range(4):
                    tb = 4 * w + i
                    for ew in range(C // W):
                        o_ps = psum.tile([P, W], F32, tag="mm",
                                         name=f"o{tb}_{ew}")
                        for ch in range(PAIRS):
                            nc.tensor.matmul(
                                o_ps[:],
                                yt_w[ch][:, i * P : (i + 1) * P],
                                wp_sb[ch][:, ew * W : (ew + 1) * W],
                                start=(ch == 0),
                                stop=(ch == PAIRS - 1),
                            )
                        o_sb = o_pool.tile([P, W], F32, tag="osb")
                        nc.vector.tensor_add(
                            o_sb[:], o_ps[:], bpr_t[:, ew * W : (ew + 1) * W]
                        )
                        nc.sync.dma_start(
                            out[tb * P : (tb + 1) * P, ew * W : (ew + 1) * W],
                            o_sb[:],
                        )

    nc.compile()
    _CACHE["nc"] = nc
    return nc


def make_in_maps(x, w_attn, b_attn, w_proj, b_proj):
    """Host-side sharding: per-core input dict."""
    x = np.ascontiguousarray(np.asarray(x, dtype=np.float32))
    w_attn = np.asarray(w_attn, dtype=np.float32)
    b_attn = np.asarray(b_attn, dtype=np.float32)
    w_proj = np.asarray(w_proj, dtype=np.float32)
    b_proj = np.asarray(b_proj, dtype=np.float32)

    trimask = np.triu(np.ones((P, P), dtype=np.float32))  # [k, q]: 1 if q >= k
    trimask2 = np.concatenate(
        [np.zeros((P, P), dtype=np.float32), trimask], axis=1
    )
    in_maps = []
    for c in range(N_CORES):
        b = c // 2
        g = c % 2
        h0 = g * HL
        # Q/K columns arranged pair-wise: [q(h0) q(h0+1) | q(h0+2) ... | k(...)]
        qcols = np.arange(h0 * D, (h0 + HL) * D)
        kcols = C + qcols
        wqk = np.concatenate(
            [w_attn[:, qcols], w_attn[:, kcols]], axis=1
        )  # [C, 1024]
        bqk_flat = np.concatenate([b_attn[qcols], b_attn[kcols]])  # [1024]
        bqk = np.ascontiguousarray(bqk_flat.reshape(2 * PAIRS, P).T)  # [128, 8]
        vcols = 2 * C + np.arange(h0 * D, (h0 + HL) * D)
        wv = np.ascontiguousarray(w_attn[:, vcols])  # [C, 512]
        bv = np.broadcast_to(b_attn[vcols], (P, HL * D)).copy()
        wp = np.ascontiguousarray(w_proj[h0 * D : (h0 + HL) * D, :])  # [512, C]
        if g == 0:
            bpr = np.broadcast_to(b_proj, (P, C)).copy()
        else:
            bpr = np.zeros((P, C), dtype=np.float32)
        in_maps.append(
            {
                "xt": np.ascontiguousarray(x[b].T),  # [C, T]
                "wqk": wqk,
                "wv": wv,
                "bqk": bqk,
                "bv": bv,
                "wp": wp,
                "bpr": bpr,
                "trimask": trimask,
                "trimask2": trimask2,
                "onesd": np.ones((P, P), dtype=np.float32),
            }
        )
    return in_maps


def kernel(x, w_attn, b_attn, w_proj, b_proj, _trace=False):
    global LAST_RESULTS
    nc = build_nc()
    in_maps = make_in_maps(x, w_attn, b_attn, w_proj, b_proj)
    res = run_bass_kernel_spmd(
        nc, in_maps, list(range(N_CORES)), trace=_trace
    )
    LAST_RESULTS = res
    outs = [res.results[c]["out"] for c in range(N_CORES)]
    y = np.stack([outs[2 * b] + outs[2 * b + 1] for b in range(B)], axis=0)
    return y.astype(np.float32)
